# revision 4
# baseline (speedup 1.0000x reference)
"""Trainium2 Bass kernel for nn_MultiHeadAttention (B=4, L=S=2048, D=1024, H=16, causal).

Sharding: 8 cores = 4 batches x 2 head-groups (8 heads each).
Per core: project its batch's q/k/v against its group's weight slices,
causal attention for 8 heads, output-projection against Wo column slice.
Host sums the 2 partial outputs per batch (tensor-parallel reduce).

v3 changes vs v2:
- All inputs arrive pre-transposed from the host: straight contiguous
  DMAs, no on-device DMA-transpose scatter.
- Q/K/V projections run as fp8e4 DoubleRow matmuls (2 cols/cycle, half
  PE energy vs bf16) with an error-compensated hi+lo split:
  X ~= Q8(32X) + Q8(32X - Q8(32X)), same for W at scale 256. Because
  e4m3 is a float, the residual reuses the same scale, so all three
  cross terms (hi*hi + hi*lo + lo*hi) accumulate in one PSUM group.
  The 2^13 scale product folds into the softmax exp scale and a
  host-prescaled Wo; net precision ~matches bf16.
- v2: row-sum matmuls folded into the ctx matmul via a ones column in V
  (65-row PSUM); causal diagonal blocks compute only surviving columns.

Attention matmuls stay bf16 with fp32 PSUM accumulation.
"""

import sys

if "/opt/trn_rl_repo" not in sys.path:
    sys.path.insert(0, "/opt/trn_rl_repo")

import numpy as np
import ml_dtypes

BF16 = ml_dtypes.bfloat16
FP8 = ml_dtypes.float8_e4m3   # TRN fp8e4: 1-4-3, max 240

# Problem constants (hardcoded per harness contract)
B, L, D, H = 4, 2048, 1024, 16
HD = D // H              # 64
NCORES = 8
GROUPS = 2               # head-groups (tensor parallel)
HG = H // GROUPS         # 8 heads per group
DG = HG * HD             # 512 out-dim per group

SX = 32.0                # fp8 scale for activations (N(0,1) -> +-176)
SW = 256.0               # fp8 scale for weights (N(0,1/1024) -> +-44)
SPROD = SX * SW          # 2^13; folded into exp scale / host Wo prescale

FULL_CFG = dict(T=L, DM=D, DG=DG)


def emit_mha(tc, aps, cfg):
    """Emit the per-core MHA program into TileContext tc."""
    import concourse.bass as bass
    from concourse import mybir

    nc = tc.nc
    f32 = mybir.dt.float32
    bf16 = mybir.dt.bfloat16
    fp8 = mybir.dt.float8e4
    DR = mybir.MatmulPerfMode.DoubleRow
    Exp = mybir.ActivationFunctionType.Exp

    T, DM, DG_ = cfg["T"], cfg["DM"], cfg["DG"]
    TB = 128                  # s/l block
    LCH = min(512, T)         # l-chunk (moving-dim)
    nDch = DM // 128          # D chunks (contraction)
    nDpr = nDch // 2          # D chunk PAIRS (DoubleRow contraction)
    nTt = T // TB             # token tiles
    nLch = T // LCH           # l-chunks
    nDiag = LCH // TB         # diagonal sub-blocks per chunk
    nPair = DG_ // 128        # head pairs (2 heads of 64 per pair)
    OCH = min(512, DM)        # Wo output chunk
    nOch = DM // OCH          # output chunks for Wo
    # scores PSUM carries SPROD^2 = 2^26; fold into the softmax scale
    SCALE = 1.0 / (np.sqrt(HD) * SPROD * SPROD)

    import contextlib

    ctx = contextlib.ExitStack()
    with ctx:
        wpool = ctx.enter_context(tc.tile_pool(name="wts", bufs=1))
        xt_pool = ctx.enter_context(tc.tile_pool(name="xt", bufs=2 * nDch))
        qkv_pool = ctx.enter_context(tc.tile_pool(name="qkv", bufs=1))
        pt_pool = ctx.enter_context(tc.tile_pool(name="pt", bufs=4))
        ctxt_pool = ctx.enter_context(tc.tile_pool(name="ctxt", bufs=1))
        small = ctx.enter_context(tc.tile_pool(name="small", bufs=2))
        outsb_pool = ctx.enter_context(tc.tile_pool(name="outsb", bufs=2))
        # PSUM: st 2x2 banks + ctx 3 + proj 1 = 8 banks
        st_ps = ctx.enter_context(tc.tile_pool(name="st_ps", bufs=2, space="PSUM"))
        ctx_ps_pool = ctx.enter_context(tc.tile_pool(name="ctx_ps", bufs=3, space="PSUM"))
        proj_ps = ctx.enter_context(tc.tile_pool(name="proj_ps", bufs=1, space="PSUM"))

        # ---- straight DMAs of host-pretransposed / pre-packed inputs ----
        def wload(nm):
            tiles = []
            for c in range(nDpr):
                hi = wpool.tile([128, 2, DG_], fp8, tag=f"{nm}h{c}")
                nc.sync.dma_start(out=hi[:], in_=aps[f"{nm}_hi"][c])
                lo = wpool.tile([128, 2, DG_], fp8, tag=f"{nm}l{c}")
                nc.sync.dma_start(out=lo[:], in_=aps[f"{nm}_lo"][c])
                tiles.append((hi, lo))
            return tiles

        def xload(nm):
            tiles = []
            for c in range(nDpr):
                hi = xt_pool.tile([128, 2, T], fp8, tag="xt")
                nc.sync.dma_start(out=hi[:], in_=aps[f"{nm}_hi"][c])
                lo = xt_pool.tile([128, 2, T], fp8, tag="xt")
                nc.sync.dma_start(out=lo[:], in_=aps[f"{nm}_lo"][c])
                tiles.append((hi, lo))
            return tiles

        # sync-queue order = consumption order
        wv = wload("wv")
        xv = xload("xv")
        # causal triangle mask for one 128x128 diagonal block, duplicated for
        # both heads of a pair: mask2[p, h, f] = (f >= p)
        mask2 = wpool.tile([128, 2, TB], bf16, tag="mask2")
        nc.sync.dma_start(out=mask2[:, 0, :], in_=aps["maskt"][:])
        nc.sync.dma_start(out=mask2[:, 1, :], in_=aps["maskt"][:])
        wq = wload("wq")
        xq = xload("xq")
        wk = wload("wk")
        xk = xload("xk")
        woT = []
        for c in range(DG_ // 128):
            t = wpool.tile([128, DM], bf16, tag=f"woT{c}")
            nc.sync.dma_start(out=t[:], in_=aps["wo"][c])
            woT.append(t)

        def proj_dr(ps, lhs_pairs, rhs_pairs, lslice, rslice):
            """out += sum over D of x*w via 3-term compensated fp8 DoubleRow."""
            n = 3 * nDpr
            i = 0
            for c in range(nDpr):
                lh, ll = lhs_pairs[c]
                rh, rl = rhs_pairs[c]
                for lt, rt in ((lh, rh), (lh, rl), (ll, rh)):
                    nc.tensor.matmul(ps, lhsT=lt[:, :, lslice], rhs=rt[:, :, rslice],
                                     start=(i == 0), stop=(i == n - 1),
                                     perf_mode=DR)
                    i += 1

        # ---- V projection: V[st] [128, 8, 65] (s on partitions; per-head 64
        # value dims + a ones column that makes the ctx matmul accumulate the
        # softmax denominator into its 65th output row) ----
        ALL = slice(0, None)
        V = []
        for st in range(nTt):
            ps = proj_ps.tile([128, HG, HD], f32, tag="proj")
            proj_dr(ps[:, :, :], xv, wv, slice(st * TB, (st + 1) * TB), ALL)
            vt = qkv_pool.tile([128, HG, HD + 1], bf16, tag=f"V{st}")
            nc.vector.tensor_copy(vt[:, :, 0:HD], ps[:, :, :])
            nc.vector.memset(vt[:, :, HD:HD + 1], 1.0)
            V.append(vt)

        # ---- QT projection: QT[m][n] [128, LCH] fine tiles so attention
        # chunks unblock as soon as their slice is projected ----
        QT = []
        for m in range(nPair):
            qts = []
            for n in range(nLch):
                ps = proj_ps.tile([128, LCH], f32, tag="proj")
                proj_dr(ps[:], wq, xq, slice(m * 128, (m + 1) * 128),
                        slice(n * LCH, (n + 1) * LCH))
                qtn = qkv_pool.tile([128, LCH], bf16, tag=f"QT{m}_{n}", name=f"QT{m}_{n}")
                nc.vector.tensor_copy(qtn[:], ps[:])
                qts.append(qtn)
            QT.append(qts)

        ctxT = [[None] * nLch for _ in range(nPair)]
        cpc_pool = ctx.enter_context(tc.tile_pool(name="cpc", bufs=8))
        kt_pool = ctx.enter_context(tc.tile_pool(name="ktp", bufs=2))
        tiny = ctx.enter_context(tc.tile_pool(name="tiny", bufs=1))
        for p in range(nPair):
            # KT[p] projection (fine tiles; slots recycle across pairs)
            ktn = []
            for n in range(nLch):
                ps = proj_ps.tile([128, LCH], f32, tag="proj")
                proj_dr(ps[:], wk, xk, slice(p * 128, (p + 1) * 128),
                        slice(n * LCH, (n + 1) * LCH))
                kt_t = kt_pool.tile([128, LCH], bf16, tag=f"KT{n}", name=f"KT{n}_{p}")
                nc.vector.tensor_copy(kt_t[:], ps[:])
                ktn.append(kt_t)
            qts = QT[p]

            # attention for this pair of heads
            pair_sums = small.tile([2 * nLch, LCH], f32, tag="psums")
            cpcs = []
            for i in range(nLch):
                nsb = (i + 1) * nDiag
                cps_a = ctx_ps_pool.tile([HD + 1, LCH], f32, tag="ctx")
                cps_b = ctx_ps_pool.tile([HD + 1, LCH], f32, tag="ctx")
                for j in range(nsb):
                    ktj = ktn[j // nDiag]
                    koff = (j % nDiag) * TB
                    r = j - nDiag * i
                    co = r * TB if r >= 0 else 0   # first surviving l column
                    sp = st_ps.tile([128, 2, LCH], f32, tag="st")
                    nc.tensor.matmul(sp[:, 0, co:],
                                     lhsT=ktj[0:64, koff:koff + TB],
                                     rhs=qts[i][0:64, co:],
                                     start=True, stop=True)
                    nc.tensor.matmul(sp[:, 1, co:],
                                     lhsT=ktj[64:128, koff:koff + TB],
                                     rhs=qts[i][64:128, co:],
                                     start=True, stop=True)
                    pt = pt_pool.tile([128, 2, LCH], bf16, tag="pt")
                    nc.scalar.activation(pt[:, :, co:], sp[:, :, co:], Exp, scale=float(SCALE))
                    if r >= 0:
                        nc.vector.tensor_mul(pt[:, :, co:co + TB], pt[:, :, co:co + TB],
                                             mask2[:, :, :])
                    st = (j == 0)
                    en = (j == nsb - 1)
                    nc.tensor.matmul(cps_a[:, co:], lhsT=V[j][:, 2 * p, :],
                                     rhs=pt[:, 0, co:], start=st, stop=en,
                                     skip_group_check=True)
                    nc.tensor.matmul(cps_b[:, co:], lhsT=V[j][:, 2 * p + 1, :],
                                     rhs=pt[:, 1, co:], start=st, stop=en,
                                     skip_group_check=True)
                # Drain both PSUM accumulators to SBUF immediately (frees the
                # banks so the next l-chunk's matmuls never stall and the PE
                # never idles into a HAM re-throttle). Normalization happens
                # off the critical path, batched per pair.
                cpc_a = cpc_pool.tile([HD + 1, LCH], f32, tag="cpc")
                nc.vector.tensor_copy(cpc_a[:], cps_a[:])
                cpc_b = cpc_pool.tile([HD + 1, LCH], f32, tag="cpc")
                nc.vector.tensor_copy(cpc_b[:], cps_b[:])
                cpcs.append((cpc_a, cpc_b))
                nc.gpsimd.dma_start(out=pair_sums[2 * i:2 * i + 1, :], in_=cpc_a[HD:HD + 1, :])
                nc.gpsimd.dma_start(out=pair_sums[2 * i + 1:2 * i + 2, :], in_=cpc_b[HD:HD + 1, :])

            # one batched exact reciprocal for the whole pair (DVE serial cost
            # is per-lane free-size, so [8, LCH] costs the same as [1, LCH])
            pair_rec = small.tile([2 * nLch, LCH], f32, tag="prec")
            nc.vector.reciprocal(pair_rec[:], pair_sums[:])
            for i in range(nLch):
                # gather the two recip rows to partition 0 (partition_broadcast
                # needs src base 0 on HW); tiny DMAs stay off the sync queue.
                cpc_a, cpc_b = cpcs[i]
                rec01 = tiny.tile([1, 2 * LCH], f32, tag="rec01")
                nc.gpsimd.dma_start(out=rec01[0:1, 0:LCH], in_=pair_rec[2 * i:2 * i + 1, :])
                nc.gpsimd.dma_start(out=rec01[0:1, LCH:2 * LCH], in_=pair_rec[2 * i + 1:2 * i + 2, :])
                rb = tiny.tile([128, 2 * LCH], f32, tag="rb")
                nc.gpsimd.partition_broadcast(rb[:], rec01[0:1, :])
                ct = ctxt_pool.tile([128, LCH], bf16, tag=f"ctxT{p}_{i}")
                nc.vector.tensor_mul(ct[0:64, :], cpc_a[0:64, :], rb[0:64, 0:LCH])
                # rb is partition-broadcast, so rows 0:64 hold the same values
                # as 64:128 — reading them keeps both SBUF inputs at base 0.
                nc.vector.tensor_mul(ct[64:128, :], cpc_b[0:64, :], rb[0:64, LCH:2 * LCH])
                ctxT[p][i] = ct

        # ---- Wo: y[lt*128:, :] = ctx @ WoT ----
        for lt in range(nTt):
            osb = outsb_pool.tile([128, DM], f32, tag="osb")
            for oc in range(nOch):
                ps = proj_ps.tile([128, OCH], f32, tag="proj")
                for dc in range(nPair):
                    lhsT = ctxT[dc][lt // nDiag][:, (lt % nDiag) * TB:(lt % nDiag) * TB + TB]
                    nc.tensor.matmul(ps[:], lhsT=lhsT,
                                     rhs=woT[dc][:, oc * OCH:(oc + 1) * OCH],
                                     start=(dc == 0), stop=(dc == nPair - 1))
                nc.vector.tensor_copy(osb[:, oc * OCH:(oc + 1) * OCH], ps[:])
            nc.sync.dma_start(out=aps["y"][lt * TB:(lt + 1) * TB, :], in_=osb[:])


def build_nc(cfg):
    """Build and compile the per-core Bass program."""
    import concourse.bacc as bacc
    import concourse.tile as tile
    from concourse import mybir

    T, DM, DG_ = cfg["T"], cfg["DM"], cfg["DG"]
    nDpr = DM // 256

    nc = bacc.Bacc("TRN2", target_bir_lowering=False, debug=False)
    f32 = mybir.dt.float32
    bf16 = mybir.dt.bfloat16
    fp8 = mybir.dt.float8e4
    aps = {}
    specs = [("maskt", [128, 128], bf16), ("wo", [DG_ // 128, 128, DM], bf16)]
    for x in ("xq", "xk", "xv"):
        for h in ("hi", "lo"):
            specs.append((f"{x}_{h}", [nDpr, 128, 2, T], fp8))
    for w in ("wq", "wk", "wv"):
        for h in ("hi", "lo"):
            specs.append((f"{w}_{h}", [nDpr, 128, 2, DG_], fp8))
    for nm, shape, dt in specs:
        aps[nm] = nc.dram_tensor(nm, shape, dt, kind="ExternalInput").ap()
    aps["y"] = nc.dram_tensor("y", [T, DM], f32, kind="ExternalOutput").ap()

    with tile.TileContext(nc) as tc:
        emit_mha(tc, aps, cfg)
    nc.compile()
    return nc


_CACHE = {}


def _get_nc():
    if "nc" not in _CACHE:
        _CACHE["nc"] = build_nc(FULL_CFG)
    return _CACHE["nc"]


def _pack_pairs(arr_t, scale):
    """[D, N] f32 -> (hi, lo) [D/256, 128, 2, N] fp8 DoubleRow pair layout."""
    d, n = arr_t.shape
    s = (arr_t * scale).astype(np.float32)
    hi = s.astype(FP8)
    lo = (s - hi.astype(np.float32)).astype(FP8)

    def pack(a):
        return np.ascontiguousarray(
            a.reshape(d // 256, 2, 128, n).transpose(0, 2, 1, 3))

    return pack(hi), pack(lo)


def shard_inputs(q, k, v, Wq, Wk, Wv, Wo):
    """Build the per-core input maps (8 cores = 4 batches x 2 groups)."""
    maskt = np.triu(np.ones((128, 128), dtype=np.float32)).astype(BF16)
    xs = {}
    for b in range(B):
        xs[("xq", b)] = _pack_pairs(np.ascontiguousarray(q[b].T), SX)
        xs[("xk", b)] = _pack_pairs(np.ascontiguousarray(k[b].T), SX)
        xs[("xv", b)] = _pack_pairs(np.ascontiguousarray(v[b].T), SX)
    ws = {}
    for g in range(GROUPS):
        rows = slice(g * DG, (g + 1) * DG)
        ws[("wq", g)] = _pack_pairs(np.ascontiguousarray(Wq[rows].T), SW)
        ws[("wk", g)] = _pack_pairs(np.ascontiguousarray(Wk[rows].T), SW)
        ws[("wv", g)] = _pack_pairs(np.ascontiguousarray(Wv[rows].T), SW)
        ws[("wo", g)] = np.ascontiguousarray(
            (Wo[:, rows].T / SPROD).astype(BF16).reshape(DG // 128, 128, D))
    in_maps = []
    for core in range(NCORES):
        b, g = divmod(core, GROUPS)
        m = {"maskt": maskt, "wo": ws[("wo", g)]}
        for x in ("xq", "xk", "xv"):
            m[f"{x}_hi"], m[f"{x}_lo"] = xs[(x, b)]
        for w in ("wq", "wk", "wv"):
            m[f"{w}_hi"], m[f"{w}_lo"] = ws[(w, g)]
        in_maps.append(m)
    return in_maps


def kernel(q, k, v, mask, Wq, Wk, Wv, Wo):
    from concourse import bass_utils

    q = np.asarray(q, dtype=np.float32)
    k = np.asarray(k, dtype=np.float32)
    v = np.asarray(v, dtype=np.float32)
    Wq = np.asarray(Wq, dtype=np.float32)
    Wk = np.asarray(Wk, dtype=np.float32)
    Wv = np.asarray(Wv, dtype=np.float32)
    Wo = np.asarray(Wo, dtype=np.float32)

    nc = _get_nc()
    in_maps = shard_inputs(q, k, v, Wq, Wk, Wv, Wo)
    res = bass_utils.run_bass_kernel_spmd(nc, in_maps, core_ids=list(range(NCORES)))
    out = np.zeros((B, L, D), dtype=np.float32)
    for core in range(NCORES):
        b = core // GROUPS
        out[b] += res.results[core]["y"]
    return out


# revision 5
# speedup vs baseline: 1.0866x; 1.0866x over previous
"""Trainium2 Bass kernel for nn_MultiHeadAttention (B=4, L=S=2048, D=1024, H=16, causal).

Sharding: 8 cores = 4 batches x 2 head-groups (8 heads each).
Per core: project its batch's q/k/v against its group's weight slices,
causal attention for 8 heads, output-projection against Wo column slice.
Host sums the 2 partial outputs per batch (tensor-parallel reduce).

v4 changes vs v2:
- All inputs arrive pre-transposed from the host: straight contiguous
  DMAs, no on-device DMA-transpose scatter (halves sync-engine load).
- Attention inner loop is software-pipelined: the ctx/denominator
  matmul for block j is emitted after the scores matmuls of block j+1,
  so the PE never stalls on the ACT exp latency. Sub-us PE gaps are
  poison twice over: the gap itself, plus the 3us continuous-busy
  requirement to reach the 2.4 GHz p-state.
- v2: row-sum matmuls folded into the ctx matmul via a ones column in V
  (65-row PSUM); causal diagonal blocks compute only surviving columns.

All matmuls in bf16 with fp32 PSUM accumulation.
"""

import sys

if "/opt/trn_rl_repo" not in sys.path:
    sys.path.insert(0, "/opt/trn_rl_repo")

import numpy as np
import ml_dtypes

BF16 = ml_dtypes.bfloat16

# Problem constants (hardcoded per harness contract)
B, L, D, H = 4, 2048, 1024, 16
HD = D // H              # 64
NCORES = 8
GROUPS = 2               # head-groups (tensor parallel)
HG = H // GROUPS         # 8 heads per group
DG = HG * HD             # 512 out-dim per group

FULL_CFG = dict(T=L, DM=D, DG=DG)


def emit_mha(tc, aps, cfg):
    """Emit the per-core MHA program into TileContext tc."""
    import concourse.bass as bass
    from concourse import mybir

    nc = tc.nc
    f32 = mybir.dt.float32
    bf16 = mybir.dt.bfloat16
    Exp = mybir.ActivationFunctionType.Exp

    T, DM, DG_ = cfg["T"], cfg["DM"], cfg["DG"]
    TB = 128                  # s/l block
    LCH = min(512, T)         # l-chunk (moving-dim)
    nDch = DM // 128          # D chunks (contraction)
    nTt = T // TB             # token tiles
    nLch = T // LCH           # l-chunks
    nDiag = LCH // TB         # diagonal sub-blocks per chunk
    nPair = DG_ // 128        # head pairs (2 heads of 64 per pair)
    OCH = min(512, DM)        # Wo output chunk
    nOch = DM // OCH          # output chunks for Wo
    SCALE = 1.0 / np.sqrt(HD)

    import contextlib

    ctx = contextlib.ExitStack()
    with ctx:
        wpool = ctx.enter_context(tc.tile_pool(name="wts", bufs=1))
        xt_pool = ctx.enter_context(tc.tile_pool(name="xt", bufs=2 * nDch))
        qkv_pool = ctx.enter_context(tc.tile_pool(name="qkv", bufs=1))
        pt_pool = ctx.enter_context(tc.tile_pool(name="pt", bufs=4))
        ctxt_pool = ctx.enter_context(tc.tile_pool(name="ctxt", bufs=1))
        small = ctx.enter_context(tc.tile_pool(name="small", bufs=2))
        outsb_pool = ctx.enter_context(tc.tile_pool(name="outsb", bufs=2))
        # PSUM: st 2x2 banks + ctx 3 + proj 1 = 8 banks
        st_ps = ctx.enter_context(tc.tile_pool(name="st_ps", bufs=2, space="PSUM"))
        ctx_ps_pool = ctx.enter_context(tc.tile_pool(name="ctx_ps", bufs=3, space="PSUM"))
        proj_ps = ctx.enter_context(tc.tile_pool(name="proj_ps", bufs=1, space="PSUM"))

        # ---- straight DMAs of host-pretransposed inputs ----
        def wload(nm, n=None):
            tiles = []
            for c in range(nDch):
                t = wpool.tile([128, DG_], bf16, tag=f"{nm}{c}")
                nc.sync.dma_start(out=t[:], in_=aps[nm][c])
                tiles.append(t)
            return tiles

        def xload(nm):
            tiles = []
            for c in range(nDch):
                t = xt_pool.tile([128, T], bf16, tag="xt")
                nc.sync.dma_start(out=t[:], in_=aps[nm][c])
                tiles.append(t)
            return tiles

        # sync-queue order = consumption order
        wvT = wload("wv")
        vT = xload("xv")
        # causal triangle mask for one 128x128 diagonal block, duplicated for
        # both heads of a pair: mask2[p, h, f] = (f >= p)
        mask2 = wpool.tile([128, 2, TB], bf16, tag="mask2")
        nc.sync.dma_start(out=mask2[:, 0, :], in_=aps["maskt"][:])
        nc.sync.dma_start(out=mask2[:, 1, :], in_=aps["maskt"][:])
        wqT = wload("wq")
        qT = xload("xq")
        wkT = wload("wk")
        kT = xload("xk")
        woT = []
        for c in range(DG_ // 128):
            t = wpool.tile([128, DM], bf16, tag=f"woT{c}")
            nc.sync.dma_start(out=t[:], in_=aps["wo"][c])
            woT.append(t)

        # ---- V projection: V[st] [128, 8, 65] (s on partitions; per-head 64
        # value dims + a ones column that makes the ctx matmul accumulate the
        # softmax denominator into its 65th output row) ----
        V = []
        for st in range(nTt):
            ps = proj_ps.tile([128, HG, HD], f32, tag="proj")
            for c in range(nDch):
                nc.tensor.matmul(ps[:, :, :], lhsT=vT[c][:, st * TB:(st + 1) * TB],
                                 rhs=wvT[c][:], start=(c == 0), stop=(c == nDch - 1))
            vt = qkv_pool.tile([128, HG, HD + 1], bf16, tag=f"V{st}")
            nc.vector.tensor_copy(vt[:, :, 0:HD], ps[:, :, :])
            nc.vector.memset(vt[:, :, HD:HD + 1], 1.0)
            V.append(vt)

        # ---- QT projection: QT[m][n] [128, LCH] fine tiles so attention
        # chunks unblock as soon as their slice is projected ----
        QT = []
        for m in range(nPair):
            qts = []
            for n in range(nLch):
                ps = proj_ps.tile([128, LCH], f32, tag="proj")
                for c in range(nDch):
                    nc.tensor.matmul(ps[:], lhsT=wqT[c][:, m * 128:(m + 1) * 128],
                                     rhs=qT[c][:, n * LCH:(n + 1) * LCH],
                                     start=(c == 0), stop=(c == nDch - 1))
                qtn = qkv_pool.tile([128, LCH], bf16, tag=f"QT{m}_{n}", name=f"QT{m}_{n}")
                nc.vector.tensor_copy(qtn[:], ps[:])
                qts.append(qtn)
            QT.append(qts)

        ctxT = [[None] * nLch for _ in range(nPair)]
        cpc_pool = ctx.enter_context(tc.tile_pool(name="cpc", bufs=8))
        kt_pool = ctx.enter_context(tc.tile_pool(name="ktp", bufs=2))
        tiny = ctx.enter_context(tc.tile_pool(name="tiny", bufs=1))
        for p in range(nPair):
            # KT[p] projection (fine tiles; slots recycle across pairs)
            ktn = []
            for n in range(nLch):
                ps = proj_ps.tile([128, LCH], f32, tag="proj")
                for c in range(nDch):
                    nc.tensor.matmul(ps[:], lhsT=wkT[c][:, p * 128:(p + 1) * 128],
                                     rhs=kT[c][:, n * LCH:(n + 1) * LCH],
                                     start=(c == 0), stop=(c == nDch - 1))
                kt_t = kt_pool.tile([128, LCH], bf16, tag=f"KT{n}", name=f"KT{n}_{p}")
                nc.vector.tensor_copy(kt_t[:], ps[:])
                ktn.append(kt_t)
            qts = QT[p]

            # attention for this pair of heads
            pair_sums = small.tile([2 * nLch, LCH], f32, tag="psums")
            cpcs = []
            for i in range(nLch):
                nsb = (i + 1) * nDiag
                cps_a = ctx_ps_pool.tile([HD + 1, LCH], f32, tag="ctx")
                cps_b = ctx_ps_pool.tile([HD + 1, LCH], f32, tag="ctx")

                def emit_ctx(j, pt, co):
                    st = (j == 0)
                    en = (j == nsb - 1)
                    nc.tensor.matmul(cps_a[:, co:], lhsT=V[j][:, 2 * p, :],
                                     rhs=pt[:, 0, co:], start=st, stop=en,
                                     skip_group_check=True)
                    nc.tensor.matmul(cps_b[:, co:], lhsT=V[j][:, 2 * p + 1, :],
                                     rhs=pt[:, 1, co:], start=st, stop=en,
                                     skip_group_check=True)

                pending = None
                for j in range(nsb):
                    ktj = ktn[j // nDiag]
                    koff = (j % nDiag) * TB
                    r = j - nDiag * i
                    co = r * TB if r >= 0 else 0   # first surviving l column
                    sp = st_ps.tile([128, 2, LCH], f32, tag="st")
                    nc.tensor.matmul(sp[:, 0, co:],
                                     lhsT=ktj[0:64, koff:koff + TB],
                                     rhs=qts[i][0:64, co:],
                                     start=True, stop=True)
                    nc.tensor.matmul(sp[:, 1, co:],
                                     lhsT=ktj[64:128, koff:koff + TB],
                                     rhs=qts[i][64:128, co:],
                                     start=True, stop=True)
                    pt = pt_pool.tile([128, 2, LCH], bf16, tag="pt")
                    nc.scalar.activation(pt[:, :, co:], sp[:, :, co:], Exp, scale=float(SCALE))
                    if r >= 0:
                        nc.vector.tensor_mul(pt[:, :, co:co + TB], pt[:, :, co:co + TB],
                                             mask2[:, :, :])
                    # ctx runs one block behind scores: while the ACT engine
                    # computes exp(j), the PE streams scores(j+1); ctx(j) then
                    # finds its pt ready and the PE never idles (sub-us PE
                    # gaps would also drop the clock out of the 2.4GHz pstate).
                    if pending is not None:
                        emit_ctx(*pending)
                    pending = (j, pt, co)
                emit_ctx(*pending)
                # Drain both PSUM accumulators to SBUF immediately (frees the
                # banks so the next l-chunk's matmuls never stall).
                # Normalization happens off the critical path, batched per pair.
                cpc_a = cpc_pool.tile([HD + 1, LCH], f32, tag="cpc")
                nc.vector.tensor_copy(cpc_a[:], cps_a[:])
                cpc_b = cpc_pool.tile([HD + 1, LCH], f32, tag="cpc")
                nc.vector.tensor_copy(cpc_b[:], cps_b[:])
                cpcs.append((cpc_a, cpc_b))
                nc.gpsimd.dma_start(out=pair_sums[2 * i:2 * i + 1, :], in_=cpc_a[HD:HD + 1, :])
                nc.gpsimd.dma_start(out=pair_sums[2 * i + 1:2 * i + 2, :], in_=cpc_b[HD:HD + 1, :])

            # one batched exact reciprocal for the whole pair (DVE serial cost
            # is per-lane free-size, so [8, LCH] costs the same as [1, LCH])
            pair_rec = small.tile([2 * nLch, LCH], f32, tag="prec")
            nc.vector.reciprocal(pair_rec[:], pair_sums[:])
            for i in range(nLch):
                # gather the two recip rows to partition 0 (partition_broadcast
                # needs src base 0 on HW); tiny DMAs stay off the sync queue.
                cpc_a, cpc_b = cpcs[i]
                rec01 = tiny.tile([1, 2 * LCH], f32, tag="rec01")
                nc.gpsimd.dma_start(out=rec01[0:1, 0:LCH], in_=pair_rec[2 * i:2 * i + 1, :])
                nc.gpsimd.dma_start(out=rec01[0:1, LCH:2 * LCH], in_=pair_rec[2 * i + 1:2 * i + 2, :])
                rb = tiny.tile([128, 2 * LCH], f32, tag="rb")
                nc.gpsimd.partition_broadcast(rb[:], rec01[0:1, :])
                ct = ctxt_pool.tile([128, LCH], bf16, tag=f"ctxT{p}_{i}")
                nc.vector.tensor_mul(ct[0:64, :], cpc_a[0:64, :], rb[0:64, 0:LCH])
                # rb is partition-broadcast, so rows 0:64 hold the same values
                # as 64:128 — reading them keeps both SBUF inputs at base 0.
                nc.vector.tensor_mul(ct[64:128, :], cpc_b[0:64, :], rb[0:64, LCH:2 * LCH])
                ctxT[p][i] = ct

        # ---- Wo: y[lt*128:, :] = ctx @ WoT ----
        for lt in range(nTt):
            osb = outsb_pool.tile([128, DM], f32, tag="osb")
            for oc in range(nOch):
                ps = proj_ps.tile([128, OCH], f32, tag="proj")
                for dc in range(nPair):
                    lhsT = ctxT[dc][lt // nDiag][:, (lt % nDiag) * TB:(lt % nDiag) * TB + TB]
                    nc.tensor.matmul(ps[:], lhsT=lhsT,
                                     rhs=woT[dc][:, oc * OCH:(oc + 1) * OCH],
                                     start=(dc == 0), stop=(dc == nPair - 1))
                nc.vector.tensor_copy(osb[:, oc * OCH:(oc + 1) * OCH], ps[:])
            nc.sync.dma_start(out=aps["y"][lt * TB:(lt + 1) * TB, :], in_=osb[:])


def build_nc(cfg):
    """Build and compile the per-core Bass program."""
    import concourse.bacc as bacc
    import concourse.tile as tile
    from concourse import mybir

    T, DM, DG_ = cfg["T"], cfg["DM"], cfg["DG"]

    nc = bacc.Bacc("TRN2", target_bir_lowering=False, debug=False)
    f32 = mybir.dt.float32
    bf16 = mybir.dt.bfloat16
    aps = {}
    specs = [
        ("maskt", [128, 128], bf16),
        ("wo", [DG_ // 128, 128, DM], bf16),
    ]
    for x in ("xq", "xk", "xv"):
        specs.append((x, [DM // 128, 128, T], bf16))
    for w in ("wq", "wk", "wv"):
        specs.append((w, [DM // 128, 128, DG_], bf16))
    for nm, shape, dt in specs:
        aps[nm] = nc.dram_tensor(nm, shape, dt, kind="ExternalInput").ap()
    aps["y"] = nc.dram_tensor("y", [T, DM], f32, kind="ExternalOutput").ap()

    with tile.TileContext(nc) as tc:
        emit_mha(tc, aps, cfg)
    nc.compile()
    return nc


_CACHE = {}


def _get_nc():
    if "nc" not in _CACHE:
        _CACHE["nc"] = build_nc(FULL_CFG)
    return _CACHE["nc"]


def _chunks(arr_t):
    """[D, N] -> [D/128, 128, N] bf16 chunk-major layout."""
    d, n = arr_t.shape
    return np.ascontiguousarray(arr_t.reshape(d // 128, 128, n).astype(BF16))


def shard_inputs(q, k, v, Wq, Wk, Wv, Wo):
    """Build the per-core input maps (8 cores = 4 batches x 2 groups)."""
    maskt = np.triu(np.ones((128, 128), dtype=np.float32)).astype(BF16)
    xs = {}
    for b in range(B):
        xs[("xq", b)] = _chunks(q[b].T)
        xs[("xk", b)] = _chunks(k[b].T)
        xs[("xv", b)] = _chunks(v[b].T)
    ws = {}
    for g in range(GROUPS):
        rows = slice(g * DG, (g + 1) * DG)
        ws[("wq", g)] = _chunks(Wq[rows].T)
        ws[("wk", g)] = _chunks(Wk[rows].T)
        ws[("wv", g)] = _chunks(Wv[rows].T)
        ws[("wo", g)] = _chunks(Wo[:, rows].T)
    in_maps = []
    for core in range(NCORES):
        b, g = divmod(core, GROUPS)
        m = {"maskt": maskt, "wo": ws[("wo", g)]}
        for x in ("xq", "xk", "xv"):
            m[x] = xs[(x, b)]
        for w in ("wq", "wk", "wv"):
            m[w] = ws[(w, g)]
        in_maps.append(m)
    return in_maps


def kernel(q, k, v, mask, Wq, Wk, Wv, Wo):
    from concourse import bass_utils

    q = np.asarray(q, dtype=np.float32)
    k = np.asarray(k, dtype=np.float32)
    v = np.asarray(v, dtype=np.float32)
    Wq = np.asarray(Wq, dtype=np.float32)
    Wk = np.asarray(Wk, dtype=np.float32)
    Wv = np.asarray(Wv, dtype=np.float32)
    Wo = np.asarray(Wo, dtype=np.float32)

    nc = _get_nc()
    in_maps = shard_inputs(q, k, v, Wq, Wk, Wv, Wo)
    res = bass_utils.run_bass_kernel_spmd(nc, in_maps, core_ids=list(range(NCORES)))
    out = np.zeros((B, L, D), dtype=np.float32)
    for core in range(NCORES):
        b = core // GROUPS
        out[b] += res.results[core]["y"]
    return out


# revision 12
# speedup vs baseline: 1.2645x; 1.1637x over previous
"""Trainium2 Bass kernel for nn_MultiHeadAttention (B=4, L=S=2048, D=1024, H=16, causal).

Sharding: 8 cores = 4 batches x 2 head-groups (8 heads each).
Per core: project its batch's q/k/v against its group's weight slices,
causal attention for 8 heads, output-projection against Wo column slice.
Host sums the 2 partial outputs per batch (tensor-parallel reduce).

v5 schedule (the attention phase is ACT(exp)-bound; everything else is
woven into its PE idle slots):
- upfront: V projection, QT[0] projection, K[0] projection (double-
  buffered PSUM via a shared pool tag -> no drain stalls).
- pair p attention: scores/ctx software-pipelined one block apart; the
  projections for pair p+1 (Q and K) are emitted as filler between
  blocks, filling the PE gap left by the exp latency. For pair 3 the
  filler is the Wo output projection of already-normalized chunks, so
  almost no PE-bound tail runs inside the worst HAM-throttle window.
- softmax normalization happens per chunk, reading ctx straight from
  PSUM (no drain copies); KT copies run on the Pool engine to keep the
  DVE path (mask muls) short.
- v2 tricks kept: row-sums folded into the ctx matmul via a ones column
  in V (65-row PSUM); causal diagonal blocks compute only surviving
  columns; all inputs host-pretransposed for straight DMAs.

All matmuls in bf16 with fp32 PSUM accumulation.
"""

import sys

if "/opt/trn_rl_repo" not in sys.path:
    sys.path.insert(0, "/opt/trn_rl_repo")

import numpy as np
import ml_dtypes

BF16 = ml_dtypes.bfloat16

# Problem constants (hardcoded per harness contract)
B, L, D, H = 4, 2048, 1024, 16
HD = D // H              # 64
NCORES = 8
GROUPS = 2               # head-groups (tensor parallel)
HG = H // GROUPS         # 8 heads per group
DG = HG * HD             # 512 out-dim per group

FULL_CFG = dict(T=L, DM=D, DG=DG)


def emit_mha(tc, aps, cfg):
    """Emit the per-core MHA program into TileContext tc."""
    import concourse.bass as bass
    from concourse import mybir

    nc = tc.nc
    f32 = mybir.dt.float32
    bf16 = mybir.dt.bfloat16
    Exp = mybir.ActivationFunctionType.Exp

    T, DM, DG_ = cfg["T"], cfg["DM"], cfg["DG"]
    TB = 128                  # s/l block
    LCH = min(512, T)         # l-chunk (moving-dim)
    nDch = DM // 128          # D chunks (contraction)
    nTt = T // TB             # token tiles
    nLch = T // LCH           # l-chunks
    nDiag = LCH // TB         # diagonal sub-blocks per chunk
    nPair = DG_ // 128        # head pairs (2 heads of 64 per pair)
    OCH = min(512, DM)        # Wo output chunk
    nOch = DM // OCH          # output chunks for Wo
    SCALE = 1.0 / np.sqrt(HD)

    import contextlib

    ctx = contextlib.ExitStack()
    with ctx:
        wpool = ctx.enter_context(tc.tile_pool(name="wts", bufs=1))
        xt_pool = ctx.enter_context(tc.tile_pool(name="xt", bufs=2 * nDch))
        qkv_pool = ctx.enter_context(tc.tile_pool(name="qkv", bufs=1))
        pt_pool = ctx.enter_context(tc.tile_pool(name="pt", bufs=4))
        ctxt_pool = ctx.enter_context(tc.tile_pool(name="ctxt", bufs=1))
        small = ctx.enter_context(tc.tile_pool(name="small", bufs=2))
        outsb_pool = ctx.enter_context(tc.tile_pool(name="outsb", bufs=2))
        kt_pool = ctx.enter_context(tc.tile_pool(name="ktp", bufs=2))
        cpc_pool = ctx.enter_context(tc.tile_pool(name="cpc", bufs=4))
        tiny = ctx.enter_context(tc.tile_pool(name="tiny", bufs=1))
        # PSUM budget (8 banks): "st" 2x2 (scores + V/Q projection) +
        # "kw" 1x2 (K proj / QT fillers / Wo, interleaved into attention) +
        # ctx 1x2
        st_ps = ctx.enter_context(tc.tile_pool(name="st_ps", bufs=2, space="PSUM"))
        kw_ps = ctx.enter_context(tc.tile_pool(name="kw_ps", bufs=2, space="PSUM"))
        ctx_ps_pool = ctx.enter_context(tc.tile_pool(name="ctx_ps", bufs=2, space="PSUM"))

        # ---- straight DMAs of host-pretransposed inputs ----
        def wload(nm):
            tiles = []
            for c in range(nDch):
                t = wpool.tile([128, DG_], bf16, tag=f"{nm}{c}")
                nc.sync.dma_start(out=t[:], in_=aps[nm][c])
                tiles.append(t)
            return tiles

        def xload(nm):
            tiles = []
            for c in range(nDch):
                t = xt_pool.tile([128, T], bf16, tag="xt")
                nc.sync.dma_start(out=t[:], in_=aps[nm][c])
                tiles.append(t)
            return tiles

        # sync-queue order = consumption order
        wvT = wload("wv")
        vT = xload("xv")
        mask2 = wpool.tile([128, 2, TB], bf16, tag="mask2")
        nc.sync.dma_start(out=mask2[:, 0, :], in_=aps["maskt"][:])
        nc.sync.dma_start(out=mask2[:, 1, :], in_=aps["maskt"][:])
        wqT = wload("wq")
        qT = xload("xq")
        wkT = wload("wk")
        kT = xload("xk")
        woT = []
        for c in range(DG_ // 128):
            t = wpool.tile([128, DM], bf16, tag=f"woT{c}")
            nc.sync.dma_start(out=t[:], in_=aps["wo"][c])
            woT.append(t)

        # ---- V projection: V[st] [128, 8, 65] (s on partitions; per-head 64
        # value dims + a ones column: the ctx matmul's 65th output row then
        # accumulates the softmax denominator for free) ----
        V = []
        for st in range(nTt):
            ps = st_ps.tile([128, HG, HD], f32, tag="st")
            for c in range(nDch):
                nc.tensor.matmul(ps[:, :, :], lhsT=vT[c][:, st * TB:(st + 1) * TB],
                                 rhs=wvT[c][:], start=(c == 0), stop=(c == nDch - 1))
            vt = qkv_pool.tile([128, HG, HD + 1], bf16, tag=f"V{st}")
            nc.vector.tensor_copy(vt[:, :, 0:HD], ps[:, :, :])
            nc.vector.memset(vt[:, :, HD:HD + 1], 1.0)
            V.append(vt)

        QT = [[None] * nLch for _ in range(nPair)]
        KT = [[None] * nLch for _ in range(nPair)]

        def emit_qt(m, n, half, ps_box):
            """Half a QT[m][n] projection (4 of 8 K-chunks)."""
            if half == 0:
                pool = st_ps if ps_box[1] else kw_ps
                tag = "st" if ps_box[1] else "kw"
                ps_box[0] = pool.tile([128, LCH], f32, tag=tag, name=f"qtps{m}_{n}")
            ps = ps_box[0]
            for c in range(half * nDch // 2, (half + 1) * nDch // 2):
                nc.tensor.matmul(ps[:], lhsT=wqT[c][:, m * 128:(m + 1) * 128],
                                 rhs=qT[c][:, n * LCH:(n + 1) * LCH],
                                 start=(c == 0), stop=(c == nDch - 1))
            if half == 1:
                qtn = qkv_pool.tile([128, LCH], bf16, tag=f"QT{m}_{n}", name=f"QT{m}_{n}")
                nc.vector.tensor_copy(qtn[:], ps[:])
                QT[m][n] = qtn

        def emit_kt(p, n, half, ps_box):
            """Half a KT[p][n] projection."""
            if half == 0:
                ps_box[0] = kw_ps.tile([128, LCH], f32, tag="kw", name=f"ktps{p}_{n}")
            ps = ps_box[0]
            for c in range(half * nDch // 2, (half + 1) * nDch // 2):
                nc.tensor.matmul(ps[:], lhsT=wkT[c][:, p * 128:(p + 1) * 128],
                                 rhs=kT[c][:, n * LCH:(n + 1) * LCH],
                                 start=(c == 0), stop=(c == nDch - 1))
            if half == 1:
                kt_t = kt_pool.tile([128, LCH], bf16, tag=f"KT{n}", name=f"KT{n}_{p}")
                nc.vector.tensor_copy(kt_t[:], ps[:])
                KT[p][n] = kt_t

        # upfront (inside the HAM free burst): QT[0], KT[0] double-buffered
        for n in range(nLch):
            box = [None, True]
            emit_qt(0, n, 0, box)
            emit_qt(0, n, 1, box)
        for n in range(nLch):
            box = [None]
            emit_kt(0, n, 0, box)
            emit_kt(0, n, 1, box)

        def proj_fillers(pnext):
            """Filler units projecting QT/KT for the next pair."""
            units = []
            for n in range(nLch):
                qbox = [None, False]
                units.append(lambda n=n, b=qbox: emit_qt(pnext, n, 0, b))
                units.append(lambda n=n, b=qbox: emit_qt(pnext, n, 1, b))
            for n in range(nLch):
                kbox = [None]
                units.append(lambda n=n, b=kbox: emit_kt(pnext, n, 0, b))
                units.append(lambda n=n, b=kbox: emit_kt(pnext, n, 1, b))
            return units

        ctxT = [[None] * nLch for _ in range(nPair)]
        osb_box = {}

        def emit_wo(lt, oc):
            """One Wo output block: y[lt, oc*OCH:] = ctx(lt) @ woT[:, oc]."""
            if oc == 0:
                osb_box[lt] = outsb_pool.tile([128, DM], f32, tag="osb", name=f"osb{lt}")
            osb = osb_box[lt]
            ps = kw_ps.tile([128, OCH], f32, tag="kw")
            for dc in range(nPair):
                lhsT = ctxT[dc][lt // nDiag][:, (lt % nDiag) * TB:(lt % nDiag) * TB + TB]
                nc.tensor.matmul(ps[:], lhsT=lhsT,
                                 rhs=woT[dc][:, oc * OCH:(oc + 1) * OCH],
                                 start=(dc == 0), stop=(dc == nPair - 1))
            nc.vector.tensor_copy(osb[:, oc * OCH:(oc + 1) * OCH], ps[:])
            if oc == nOch - 1:
                nc.sync.dma_start(out=aps["y"][lt * TB:(lt + 1) * TB, :], in_=osb[:])

        for p in range(nPair):
            fillers = proj_fillers(p + 1) if p + 1 < nPair else []
            fi = 0
            qts = QT[p]
            ktn = KT[p]
            for i in range(nLch):
                nsb = (i + 1) * nDiag
                cps_a = ctx_ps_pool.tile([HD + 1, LCH], f32, tag="ctx")
                cps_b = ctx_ps_pool.tile([HD + 1, LCH], f32, tag="ctx")

                def emit_ctx(j, pt, co):
                    st = (j == 0)
                    en = (j == nsb - 1)
                    nc.tensor.matmul(cps_a[:, co:], lhsT=V[j][:, 2 * p, :],
                                     rhs=pt[:, 0, co:], start=st, stop=en,
                                     skip_group_check=True)
                    nc.tensor.matmul(cps_b[:, co:], lhsT=V[j][:, 2 * p + 1, :],
                                     rhs=pt[:, 1, co:], start=st, stop=en,
                                     skip_group_check=True)

                pending = None
                for j in range(nsb):
                    ktj = ktn[j // nDiag]
                    koff = (j % nDiag) * TB
                    r = j - nDiag * i
                    co = r * TB if r >= 0 else 0   # first surviving l column
                    sp = st_ps.tile([128, 2, LCH], f32, tag="st")
                    nc.tensor.matmul(sp[:, 0, co:],
                                     lhsT=ktj[0:64, koff:koff + TB],
                                     rhs=qts[i][0:64, co:],
                                     start=True, stop=True)
                    nc.tensor.matmul(sp[:, 1, co:],
                                     lhsT=ktj[64:128, koff:koff + TB],
                                     rhs=qts[i][64:128, co:],
                                     start=True, stop=True)
                    pt = pt_pool.tile([128, 2, LCH], bf16, tag="pt")
                    nc.scalar.activation(pt[:, :, co:], sp[:, :, co:], Exp, scale=float(SCALE))
                    if r >= 0:
                        nc.vector.tensor_mul(pt[:, :, co:co + TB], pt[:, :, co:co + TB],
                                             mask2[:, :, :])
                    # ctx runs one block behind scores so the PE never waits
                    # on the exp; filler (next pair's projections / Wo) soaks
                    # up the remaining ACT-bound slack.
                    if pending is not None:
                        emit_ctx(*pending)
                    pending = (j, pt, co)
                    if fi < len(fillers):
                        fillers[fi]()
                        fi += 1
                emit_ctx(*pending)
                # ---- per-chunk softmax normalization ----
                # drains release the ctx PSUM banks fast (the release gates
                # the next chunk's first ctx matmul, bufs=2); the
                # recip/broadcast chain then runs off the critical path.
                cpc_a = cpc_pool.tile([HD + 1, LCH], f32, tag="cpc")
                nc.vector.tensor_copy(cpc_a[:], cps_a[:])
                cpc_b = cpc_pool.tile([HD + 1, LCH], f32, tag="cpc")
                nc.vector.tensor_copy(cpc_b[:], cps_b[:])
                srec = small.tile([1, 2 * LCH], f32, tag="srec")
                nc.gpsimd.dma_start(out=srec[0:1, 0:LCH], in_=cpc_a[HD:HD + 1, :])
                nc.gpsimd.dma_start(out=srec[0:1, LCH:2 * LCH], in_=cpc_b[HD:HD + 1, :])
                rec01 = tiny.tile([1, 2 * LCH], f32, tag="rec01")
                nc.vector.reciprocal(rec01[:], srec[:])
                rb = tiny.tile([128, 2 * LCH], f32, tag="rb")
                nc.gpsimd.partition_broadcast(rb[:], rec01[0:1, :])
                ct = ctxt_pool.tile([128, LCH], bf16, tag=f"ctxT{p}_{i}")
                nc.vector.tensor_mul(ct[0:64, :], cpc_a[0:64, :], rb[0:64, 0:LCH])
                # rb is partition-broadcast: rows 0:64 match 64:128, keeping
                # both SBUF inputs at base partition 0.
                nc.vector.tensor_mul(ct[64:128, :], cpc_b[0:64, :], rb[0:64, LCH:2 * LCH])
                ctxT[p][i] = ct
                if p == nPair - 1 and i + 1 < nLch:
                    # pair 3: chunk i is fully normalized -> its Wo blocks
                    # become filler for chunk i+1's attention
                    for lt in range(i * nDiag, (i + 1) * nDiag):
                        for oc in range(nOch):
                            fillers.append(lambda lt=lt, oc=oc: emit_wo(lt, oc))
            while fi < len(fillers):
                fillers[fi]()
                fi += 1

        # Wo for the final chunk
        for lt in range((nLch - 1) * nDiag, nTt):
            for oc in range(nOch):
                emit_wo(lt, oc)


def build_nc(cfg):
    """Build and compile the per-core Bass program."""
    import concourse.bacc as bacc
    import concourse.tile as tile
    from concourse import mybir

    T, DM, DG_ = cfg["T"], cfg["DM"], cfg["DG"]

    nc = bacc.Bacc("TRN2", target_bir_lowering=False, debug=False)
    f32 = mybir.dt.float32
    bf16 = mybir.dt.bfloat16
    aps = {}
    specs = [
        ("maskt", [128, 128], bf16),
        ("wo", [DG_ // 128, 128, DM], bf16),
    ]
    for x in ("xq", "xk", "xv"):
        specs.append((x, [DM // 128, 128, T], bf16))
    for w in ("wq", "wk", "wv"):
        specs.append((w, [DM // 128, 128, DG_], bf16))
    for nm, shape, dt in specs:
        aps[nm] = nc.dram_tensor(nm, shape, dt, kind="ExternalInput").ap()
    aps["y"] = nc.dram_tensor("y", [T, DM], f32, kind="ExternalOutput").ap()

    with tile.TileContext(nc) as tc:
        emit_mha(tc, aps, cfg)
    nc.compile()
    return nc


_CACHE = {}


def _get_nc():
    if "nc" not in _CACHE:
        _CACHE["nc"] = build_nc(FULL_CFG)
    return _CACHE["nc"]


def _chunks(arr_t):
    """[D, N] -> [D/128, 128, N] bf16 chunk-major layout."""
    d, n = arr_t.shape
    return np.ascontiguousarray(arr_t.reshape(d // 128, 128, n).astype(BF16))


def shard_inputs(q, k, v, Wq, Wk, Wv, Wo):
    """Build the per-core input maps (8 cores = 4 batches x 2 groups)."""
    maskt = np.triu(np.ones((128, 128), dtype=np.float32)).astype(BF16)
    xs = {}
    for b in range(B):
        xs[("xq", b)] = _chunks(q[b].T)
        xs[("xk", b)] = _chunks(k[b].T)
        xs[("xv", b)] = _chunks(v[b].T)
    ws = {}
    for g in range(GROUPS):
        rows = slice(g * DG, (g + 1) * DG)
        ws[("wq", g)] = _chunks(Wq[rows].T)
        ws[("wk", g)] = _chunks(Wk[rows].T)
        ws[("wv", g)] = _chunks(Wv[rows].T)
        ws[("wo", g)] = _chunks(Wo[:, rows].T)
    in_maps = []
    for core in range(NCORES):
        b, g = divmod(core, GROUPS)
        m = {"maskt": maskt, "wo": ws[("wo", g)]}
        for x in ("xq", "xk", "xv"):
            m[x] = xs[(x, b)]
        for w in ("wq", "wk", "wv"):
            m[w] = ws[(w, g)]
        in_maps.append(m)
    return in_maps


def kernel(q, k, v, mask, Wq, Wk, Wv, Wo):
    from concourse import bass_utils

    q = np.asarray(q, dtype=np.float32)
    k = np.asarray(k, dtype=np.float32)
    v = np.asarray(v, dtype=np.float32)
    Wq = np.asarray(Wq, dtype=np.float32)
    Wk = np.asarray(Wk, dtype=np.float32)
    Wv = np.asarray(Wv, dtype=np.float32)
    Wo = np.asarray(Wo, dtype=np.float32)

    nc = _get_nc()
    in_maps = shard_inputs(q, k, v, Wq, Wk, Wv, Wo)
    res = bass_utils.run_bass_kernel_spmd(nc, in_maps, core_ids=list(range(NCORES)))
    out = np.zeros((B, L, D), dtype=np.float32)
    for core in range(NCORES):
        b = core // GROUPS
        out[b] += res.results[core]["y"]
    return out
